# revision 3
# baseline (speedup 1.0000x reference)
"""Trainium2 Bass kernel for the two-branch sparse-attention fusion module.

Math (per batch b, tokens T = rgb/evt as (d=256, N=4096) d-major):
    s      = sum_n T[:, n]                           (256,)
    value[n] = T[:,n].v + c, v = (Wq^T Wk) s + N Wq^T bk, c = (Wk^T bq).s + N bq.bk
    w      = sigmoid((value_rgb - value_evt)/sqrt(d))
    out    = evt + w * (rgb - evt)

Dataflow (fp16 compute, fp32 DRAM I/O + PSUM accumulation):
    SP HWDGE   : weight/bias loads (f32, no cast) -> precompute starts early
    gpsimd     : casting token loads f32->fp16, partition_broadcast of the
                 sigmoid row to 128 partitions, casting stores fp16->f32
    ScalarE    : streaming row-sum partials of A, sigmoid (psv->fp16 row)
    PE         : weight-product precompute, per-batch head matvecs, fp16
                 value matmuls (no hi/lo split; rel err ~1.3e-3 << 2e-2)
    DVE        : M = A-D with rowsum accum, blend M*=wb, A=M+D (fp16 2x)

Chunk-granular software pipeline; emission order per engine tracks data
arrival so no engine head-of-line blocks. Sharded data-parallel over
batch: 8 cores x 2 batches, weights replicated.
"""

import numpy as np
from contextlib import ExitStack

import concourse.bass as bass
import concourse.tile as tile
from concourse import bacc, mybir
from concourse.bass_utils import run_bass_kernel_spmd

F32 = mybir.dt.float32
FP16 = mybir.dt.float16

BS, DIM, HH, WW = 16, 256, 64, 64
N = HH * WW                 # 4096 tokens
NCORES = 8
BPC = BS // NCORES          # batches per core
PH = DIM // 128             # partition halves of the d dim
CH = 512                    # value-chunk (one PSUM bank of f32)
NCH = N // CH               # 8
LB = 2048                   # load block columns (1 MiB DRAM-side)
NLB = N // LB               # 2
INV_SQRT_D = 1.0 / 16.0


def build_nc() -> bass.Bass:
    nc = bacc.Bacc()

    rgb = nc.declare_dram_parameter("rgb", [BPC, PH, 128, N], F32, isOutput=False)
    evt = nc.declare_dram_parameter("evt", [BPC, PH, 128, N], F32, isOutput=False)
    wts = {}
    for nm in ("Wq_a", "Wk_a", "Wq_d", "Wk_d"):
        wts[nm] = nc.declare_dram_parameter(nm, [PH, 128, DIM], F32, isOutput=False)
    bss = {}
    for nm in ("bq_a", "bk_a", "bq_d", "bk_d"):
        bss[nm] = nc.declare_dram_parameter(nm, [PH, 128, 1], F32, isOutput=False)
    out = nc.declare_dram_parameter("out", [BPC, PH, 128, N], F32, isOutput=True)

    with tile.TileContext(nc) as tc:
        _body(tc, rgb, evt, wts, bss, out)
    nc.finalize()
    return nc


def _precompute(tc, consts, ps_pre, W, B):
    """Weight products from f32 weights; the d branch carries a folded minus
    sign. PT and R are stored fp16 (they feed fp16 matvecs); U stays f32."""
    nc = tc.nc
    PT, U, R = {}, {}, {}
    for br, wq, wk, sign in (
        ("a", "Wq_a", "Wk_a", 1.0),
        ("d", "Wq_d", "Wk_d", -1.0),
    ):
        for jh in range(PH):
            ps = ps_pre.tile([128, DIM], F32, tag="ps_pre", name=f"psPT{br}{jh}")
            for oh in range(PH):
                nc.tensor.matmul(
                    ps,
                    lhsT=W[(wk, oh)][:, jh * 128 : (jh + 1) * 128],
                    rhs=W[(wq, oh)],
                    start=(oh == 0),
                    stop=(oh == PH - 1),
                )
            t = consts.tile([128, DIM], FP16, tag=f"PT{br}{jh}", name=f"PT{br}{jh}")
            nc.vector.tensor_scalar_mul(out=t, in0=ps, scalar1=sign)
            PT[(br, jh)] = t
        ps = ps_pre.tile([128, 2 * PH], F32, tag="ps_pre", name=f"psUR{br}")
        for ih in range(PH):
            for oh in range(PH):
                nc.tensor.matmul(
                    ps[:, ih : ih + 1],
                    lhsT=W[(wq, oh)][:, ih * 128 : (ih + 1) * 128],
                    rhs=B[("bk_" + br, oh)],
                    start=(oh == 0),
                    stop=(oh == PH - 1),
                )
        for jh in range(PH):
            for oh in range(PH):
                nc.tensor.matmul(
                    ps[:, PH + jh : PH + jh + 1],
                    lhsT=W[(wk, oh)][:, jh * 128 : (jh + 1) * 128],
                    rhs=B[("bq_" + br, oh)],
                    start=(oh == 0),
                    stop=(oh == PH - 1),
                )
        tU = consts.tile([128, PH], F32, tag=f"U{br}", name=f"U{br}")
        nc.vector.tensor_scalar_mul(out=tU, in0=ps[:, 0:PH], scalar1=float(sign * N))
        tR = consts.tile([128, PH], FP16, tag=f"R{br}", name=f"R{br}")
        nc.vector.tensor_scalar_mul(out=tR, in0=ps[:, PH : 2 * PH], scalar1=sign)
        U[("full", br)] = tU
        R[("full", br)] = tR

    # batch-independent bias-dot part of c_diff: N*(bq_a.bk_a - bq_d.bk_d)
    ps = ps_pre.tile([1, 1], F32, tag="ps_pre", name="psCb")
    k = 0
    for bq, bk, sgn in (("bq_a", "bk_a", 1), ("bq_d", "bk_d", -1)):
        for oh in range(PH):
            t = consts.tile([128, 1], F32, tag=f"bkN{bk}{oh}", name=f"bkN{bk}{oh}")
            nc.vector.tensor_scalar_mul(out=t, in0=B[(bk, oh)], scalar1=float(sgn * N))
            nc.tensor.matmul(ps, lhsT=B[(bq, oh)], rhs=t, start=(k == 0), stop=(k == 3))
            k += 1
    c_bias = consts.tile([1, 1], FP16, tag="c_bias")
    nc.vector.tensor_scalar_mul(out=c_bias, in0=ps, scalar1=1.0)
    return PT, U, R, c_bias


def _body(tc, rgb, evt, wts, bss, out):
    nc = tc.nc
    ACT = mybir.ActivationFunctionType
    with ExitStack() as ctx:
        consts = ctx.enter_context(tc.tile_pool(name="consts", bufs=1))
        data = ctx.enter_context(tc.tile_pool(name="data", bufs=2))
        mpool = ctx.enter_context(tc.tile_pool(name="mpool", bufs=2))
        wbp = ctx.enter_context(tc.tile_pool(name="wbp", bufs=2))
        small = ctx.enter_context(tc.tile_pool(name="small", bufs=2))
        wrp = ctx.enter_context(tc.tile_pool(name="wrp", bufs=4))
        ps_val = ctx.enter_context(tc.tile_pool(name="ps_val", bufs=4, space="PSUM"))
        ps_pre = ctx.enter_context(tc.tile_pool(name="ps_pre", bufs=2, space="PSUM"))
        ps_head = ctx.enter_context(tc.tile_pool(name="ps_head", bufs=2, space="PSUM"))

        one_one = consts.tile([1, 1], FP16, tag="one_one")
        nc.vector.memset(one_one, 1.0)
        garbage = consts.tile([128, 1], F32, tag="garbage")

        def emit_weight_loads():
            # f32 loads on the SP HWDGE queues: lands early, zero gpsimd cost
            W, B = {}, {}
            for nm in ("Wq_a", "Wk_a", "Wq_d", "Wk_d"):
                t = consts.tile([128, PH * DIM], F32, tag=nm, name=f"W{nm}")
                nc.sync.dma_start(
                    out=t.rearrange("p (h c) -> p h c", h=PH),
                    in_=wts[nm].rearrange("h p c -> p h c"),
                )
                for h in range(PH):
                    W[(nm, h)] = t[:, h * DIM : (h + 1) * DIM]
            for nm in ("bq_a", "bk_a", "bq_d", "bk_d"):
                t = consts.tile([128, PH], F32, tag=f"b{nm}", name=f"b{nm}")
                nc.sync.dma_start(
                    out=t.rearrange("p (h c) -> p h c", h=PH),
                    in_=bss[nm].rearrange("h p c -> p h c"),
                )
                for h in range(PH):
                    B[(nm, h)] = t[:, h : h + 1]
            return W, B

        st = [dict() for _ in range(BPC)]

        def emit_loads(b):
            # casting SWDGE loads: DRAM f32 -> SBUF fp16, one block at a time
            # in (blk, A/D x half) order matching downstream consumption.
            A, Dv = {}, {}
            for h in range(PH):
                A[h] = data.tile([128, N], FP16, tag=f"A{h}", name=f"A{h}_{b}")
                Dv[h] = data.tile([128, N], FP16, tag=f"D{h}", name=f"D{h}_{b}")
            for blk in range(NLB):
                sl = slice(blk * LB, (blk + 1) * LB)
                for h in range(PH):
                    nc.gpsimd.dma_start(out=A[h][:, sl], in_=rgb[b, h][:, sl])
                    nc.gpsimd.dma_start(out=Dv[h][:, sl], in_=evt[b, h][:, sl])
            st[b].update(A=A, Dv=Dv)

        def make_stage1(b):
            # per-block ops, returned as thunk lists in data-arrival order:
            #   red[i] : scalar row-sum partial of A     (blk-major, h-minor)
            #   sub[i] : DVE M = A - D with rowsum accum (blk-major, h-minor)
            A, Dv = st[b]["A"], st[b]["Dv"]
            M = {}
            sa4 = small.tile([128, 2 * NLB * PH], F32, tag="sa4", name=f"sa4_{b}")
            sm4 = small.tile([128, 2 * NLB * PH], F32, tag="sm4", name=f"sm4_{b}")
            sa16 = small.tile([128, NLB * PH], FP16, tag="sa16", name=f"sa16_{b}")
            sd16 = small.tile([128, NLB * PH], FP16, tag="sd16", name=f"sd16_{b}")
            for h in range(PH):
                M[h] = mpool.tile([128, N], FP16, tag=f"M{h}", name=f"M{h}_{b}")
            st[b].update(M=M, sa16=sa16, sd16=sd16)

            def red(h, blk):
                sl = slice(blk * LB, (blk + 1) * LB)
                nc.scalar.activation(
                    out=garbage.broadcast_to([128, LB]),
                    in_=A[h][:, sl],
                    func=ACT.Copy,
                    accum_out=sa4[:, blk * PH + h : blk * PH + h + 1],
                )

            def sub(h, blk):
                sl = slice(blk * LB, (blk + 1) * LB)
                nc.vector.scalar_tensor_tensor(
                    out=M[h][:, sl],
                    in0=A[h][:, sl],
                    scalar=1.0,
                    in1=Dv[h][:, sl],
                    op0=mybir.AluOpType.mult,
                    op1=mybir.AluOpType.subtract,
                    accum_out=sm4[:, blk * PH + h : blk * PH + h + 1],
                )

            def derive():
                # fp16 partials: sa16 = cast(sa4), sd16 = sa4 - sm4
                with nc.allow_low_precision(reason="tiny fp16 partials"):
                    nc.vector.tensor_scalar_mul(
                        out=sa16, in0=sa4[:, 0 : NLB * PH], scalar1=1.0
                    )
                    nc.vector.tensor_sub(
                        out=sd16, in0=sa4[:, 0 : NLB * PH], in1=sm4[:, 0 : NLB * PH]
                    )

            reds = [lambda h=h, blk=blk: red(h, blk) for blk in range(NLB) for h in range(PH)]
            subs = [lambda h=h, blk=blk: sub(h, blk) for blk in range(NLB) for h in range(PH)]
            return reds, subs, derive

        def head_pe(b):
            # c_diff = sum_j r[j] s[j] (both branches) + c_bias, and
            # v = PT @ s + U per branch -- all small PE matvecs
            sa16, sd16 = st[b]["sa16"], st[b]["sd16"]
            S4 = {"a": sa16, "d": sd16}
            ps_c = ps_head.tile([1, 1], F32, tag="ps_h", name=f"psc_{b}")
            terms = [
                (S4[br][:, blk * PH + jh : blk * PH + jh + 1], R[("full", br)][:, jh : jh + 1])
                for br in ("a", "d")
                for jh in range(PH)
                for blk in range(NLB)
            ]
            for i, (l, r) in enumerate(terms):
                nc.tensor.matmul(ps_c, lhsT=l, rhs=r, start=(i == 0), stop=False)
            nc.tensor.matmul(ps_c, lhsT=c_bias, rhs=one_one, start=False, stop=True)
            psv = {}
            for br in ("a", "d"):
                ps = ps_head.tile([128, PH], F32, tag="ps_h", name=f"psv{br}_{b}")
                for ih in range(PH):
                    k = 0
                    for jh in range(PH):
                        for blk in range(NLB):
                            nc.tensor.matmul(
                                ps[:, ih : ih + 1],
                                lhsT=PT[(br, jh)][:, ih * 128 : (ih + 1) * 128],
                                rhs=S4[br][:, blk * PH + jh : blk * PH + jh + 1],
                                start=(k == 0),
                                stop=(k == PH * NLB - 1),
                            )
                            k += 1
                psv[br] = ps
            st[b]["ps_c"], st[b]["ps_v"] = ps_c, psv

        def head_fin(b):
            # c16 on scalar; v add-U + fp16 cast on DVE
            c16 = small.tile([1, 1], F32, tag="c16", name=f"c16_{b}")
            nc.scalar.mul(out=c16, in_=st[b]["ps_c"], mul=INV_SQRT_D)
            VH = {}
            for br in ("a", "d"):
                v = small.tile([128, PH], F32, tag=f"v{br}", name=f"v{br}_{b}")
                nc.vector.tensor_add(out=v, in0=st[b]["ps_v"][br], in1=U[("full", br)])
                vh = small.tile([128, PH], FP16, tag=f"vh{br}", name=f"vh{br}_{b}")
                with nc.allow_low_precision(reason="fp16 matvec vector"):
                    nc.vector.tensor_scalar_mul(out=vh, in0=v, scalar1=1.0)
                VH[br] = vh
            st[b]["VH"], st[b]["c16"] = VH, c16

        def make_stage2(b):
            # per 512-chunk: PE 4 matvecs -> scalar sigmoid -> gpsimd bcast
            A, Dv = st[b]["A"], st[b]["Dv"]
            wb_sb = wbp.tile([128, N], FP16, tag="wb_sb", name=f"wb_sb_{b}")
            st[b]["wb_sb"] = wb_sb
            wrows = {}

            def chunk_pe(ich):
                VH = st[b]["VH"]
                sl = slice(ich * CH, (ich + 1) * CH)
                psv = ps_val.tile([1, CH], F32, tag="psv", name=f"psval{ich}_{b}")
                mms = [
                    (VH["a"][:, 0:1], A[0]), (VH["a"][:, 1:2], A[1]),
                    (VH["d"][:, 0:1], Dv[0]), (VH["d"][:, 1:2], Dv[1]),
                ]
                for i, (v, t) in enumerate(mms):
                    nc.tensor.matmul(
                        psv, lhsT=v, rhs=t[:, sl],
                        start=(i == 0), stop=(i == len(mms) - 1),
                    )
                wrows[ich] = psv

            def chunk_sig(ich):
                wrow = wrp.tile([1, CH], FP16, tag="wr", name=f"wrow{ich}_{b}")
                nc.scalar.activation(
                    out=wrow, in_=wrows[ich],
                    func=ACT.Sigmoid, bias=st[b]["c16"], scale=INV_SQRT_D,
                )
                wrows[ich] = wrow

            def chunk_bc(ich):
                nc.gpsimd.partition_broadcast(
                    wb_sb[:, ich * CH : (ich + 1) * CH], wrows[ich]
                )

            return chunk_pe, chunk_sig, chunk_bc

        def make_blend(b):
            # per (blk, h): DVE M *= wb ; A = M + D  (in place, all fp16 2x)
            A, Dv, M = st[b]["A"], st[b]["Dv"], st[b]["M"]

            def mul(h, blk):
                sl = slice(blk * LB, (blk + 1) * LB)
                wb_sb = st[b]["wb_sb"]
                nc.vector.tensor_mul(out=M[h][:, sl], in0=M[h][:, sl], in1=wb_sb[:, sl])

            def add(h, blk):
                sl = slice(blk * LB, (blk + 1) * LB)
                nc.vector.tensor_add(out=A[h][:, sl], in0=M[h][:, sl], in1=Dv[h][:, sl])

            def store(h, blk):
                sl = slice(blk * LB, (blk + 1) * LB)
                nc.gpsimd.dma_start(out=out[b, h][:, sl], in_=A[h][:, sl])

            return mul, add, store

        # ---- emission schedule ----------------------------------------
        emit_weight_loads_ret = emit_weight_loads()
        W, B = emit_weight_loads_ret
        emit_loads(0)
        emit_loads(1)
        PT, U, R, c_bias = _precompute(tc, consts, ps_pre, W, B)

        red0, sub0, derive0 = make_stage1(0)
        red1, sub1, derive1 = make_stage1(1)

        # scalar: reds in arrival order (b0 then b1 interleaved w/ sigmoids later)
        # DVE: subs b0
        for i in range(2 * NLB):
            red0[i]()
            sub0[i]()
        derive0()
        head_pe(0)
        head_fin(0)

        pe0, sig0, bc0 = make_stage2(0)
        mul0, add0, st0 = make_blend(0)
        pe1, sig1, bc1 = None, None, None

        # b1 stage-1 interleaved with b0 stage-2 in expected readiness order.
        # scalar stream: red1 blocks between sigmoid0 chunks
        # DVE stream: sub1 blocks around blend0 col-blocks
        # gpsimd stream: loads already queued; bcast0 chunks then stores
        red1[0]()                     # A0 blk0 of b1
        for ich in range(2):
            pe0(ich); sig0(ich); bc0(ich)
        red1[1]()                     # A1 blk0
        sub1[0](); sub1[1]()          # M blk0 (h0, h1)
        for ich in range(2, 4):
            pe0(ich); sig0(ich); bc0(ich)
        red1[2]()                     # A0 blk1
        # blend0 col-block 0 (stores emitted after bcast 4,5 to keep the
        # gpsimd stream in readiness order)
        mul0(0, 0); add0(0, 0)
        mul0(1, 0); add0(1, 0)
        for ich in range(4, 6):
            pe0(ich); sig0(ich); bc0(ich)
        st0(0, 0); st0(1, 0)
        red1[3]()                     # A1 blk1
        sub1[2](); sub1[3]()          # M blk1 (h0, h1)
        for ich in range(6, 8):
            pe0(ich); sig0(ich); bc0(ich)
        derive1()
        head_pe(1)
        head_fin(1)

        pe1, sig1, bc1 = make_stage2(1)
        mul1, add1, st1 = make_blend(1)

        # blend0 col-block 1 (wb0 complete) while b1 head/values spin up
        mul0(0, 1); add0(0, 1); st0(0, 1)
        mul0(1, 1); add0(1, 1); st0(1, 1)

        for ich in range(NCH):
            pe1(ich); sig1(ich); bc1(ich)
            if ich == 5:
                mul1(0, 0); add1(0, 0); st1(0, 0)
                mul1(1, 0); add1(1, 0); st1(1, 0)
        mul1(0, 1); add1(0, 1); st1(0, 1)
        mul1(1, 1); add1(1, 1); st1(1, 1)


_NC_CACHE = None


def _get_nc():
    global _NC_CACHE
    if _NC_CACHE is None:
        _NC_CACHE = build_nc()
    return _NC_CACHE


def _make_in_maps(inputs):
    rgb = np.ascontiguousarray(np.asarray(inputs["rgb"], dtype=np.float32)).reshape(
        BS, PH, 128, N
    )
    evt = np.ascontiguousarray(np.asarray(inputs["evt"], dtype=np.float32)).reshape(
        BS, PH, 128, N
    )
    base = {}
    for nm in ("Wq_a", "Wk_a", "Wq_d", "Wk_d"):
        base[nm] = np.ascontiguousarray(
            np.asarray(inputs[nm], dtype=np.float32)
        ).reshape(PH, 128, DIM)
    for nm in ("bq_a", "bk_a", "bq_d", "bk_d"):
        base[nm] = np.ascontiguousarray(
            np.asarray(inputs[nm], dtype=np.float32)
        ).reshape(PH, 128, 1)
    in_maps = []
    for c in range(NCORES):
        m = dict(base)
        m["rgb"] = np.ascontiguousarray(rgb[c * BPC : (c + 1) * BPC])
        m["evt"] = np.ascontiguousarray(evt[c * BPC : (c + 1) * BPC])
        in_maps.append(m)
    return in_maps


def run(inputs, trace=False):
    nc = _get_nc()
    in_maps = _make_in_maps(inputs)
    res = run_bass_kernel_spmd(nc, in_maps, core_ids=list(range(NCORES)), trace=trace)
    outs = [
        np.asarray(res.results[i]["out"]).reshape(BPC, DIM, HH, WW)
        for i in range(NCORES)
    ]
    full = np.concatenate(outs, axis=0)
    return full, res


def kernel(**inputs) -> np.ndarray:
    full, _ = run(inputs, trace=False)
    return full


# revision 4
# speedup vs baseline: 1.1772x; 1.1772x over previous
"""Trainium2 Bass kernel for the two-branch sparse-attention fusion module.

Math (per batch b, tokens T = rgb/evt as (d=256, N=4096) d-major):
    s      = sum_n T[:, n]                           (256,)
    value[n] = T[:,n].v + c, v = (Wq^T Wk) s + N Wq^T bk, c = (Wk^T bq).s + N bq.bk
    w      = sigmoid((value_rgb - value_evt)/sqrt(d))
    out    = evt + w * (rgb - evt)

Dataflow (fp16 compute, fp32 DRAM I/O + PSUM accumulation):
    SP HWDGE   : weight/bias loads (f32, no cast) -> precompute starts early
    gpsimd     : casting token loads f32->fp16, partition_broadcast of the
                 sigmoid row to 128 partitions, casting stores fp16->f32
    ScalarE    : streaming row-sum partials of A, sigmoid (psv->fp16 row)
    PE         : weight-product precompute, per-batch head matvecs, fp16
                 value matmuls (no hi/lo split; rel err ~1.3e-3 << 2e-2)
    DVE        : M = A-D with rowsum accum, blend M*=wb, A=M+D (fp16 2x)

Chunk-granular software pipeline; emission order per engine tracks data
arrival so no engine head-of-line blocks. Sharded data-parallel over
batch: 8 cores x 2 batches, weights replicated.
"""

import numpy as np
from contextlib import ExitStack

import concourse.bass as bass
import concourse.tile as tile
from concourse import bacc, mybir
from concourse.bass_utils import run_bass_kernel_spmd

F32 = mybir.dt.float32
FP16 = mybir.dt.float16

BS, DIM, HH, WW = 16, 256, 64, 64
N = HH * WW                 # 4096 tokens
NCORES = 8
BPC = BS // NCORES          # batches per core
PH = DIM // 128             # partition halves of the d dim
CH = 512                    # value-chunk (one PSUM bank of f32)
NCH = N // CH               # 8
LB = 2048                   # load block columns (1 MiB DRAM-side)
NLB = N // LB               # 2
INV_SQRT_D = 1.0 / 16.0


def build_nc() -> bass.Bass:
    nc = bacc.Bacc()

    rgb = nc.declare_dram_parameter("rgb", [BPC, PH, 128, N], F32, isOutput=False)
    evt = nc.declare_dram_parameter("evt", [BPC, PH, 128, N], F32, isOutput=False)
    # host-side packed weights/biases: one contiguous f32 row per partition
    # so the SP HWDGE load is a single clean 128-descriptor DMA each
    wpack = nc.declare_dram_parameter("wpack", [128, 4 * PH * DIM], F32, isOutput=False)
    bpack = nc.declare_dram_parameter("bpack", [128, 4 * PH], F32, isOutput=False)
    out = nc.declare_dram_parameter("out", [BPC, PH, 128, N], F32, isOutput=True)

    with tile.TileContext(nc) as tc:
        _body(tc, rgb, evt, wpack, bpack, out)
    nc.finalize()
    return nc


def _precompute(tc, consts, ps_pre, W, B):
    """Weight products from f32 weights; the d branch carries a folded minus
    sign. PT and R are stored fp16 (they feed fp16 matvecs); U stays f32."""
    nc = tc.nc
    PT, U, R = {}, {}, {}
    for br, wq, wk, sign in (
        ("a", "Wq_a", "Wk_a", 1.0),
        ("d", "Wq_d", "Wk_d", -1.0),
    ):
        for jh in range(PH):
            ps = ps_pre.tile([128, DIM], F32, tag="ps_pre", name=f"psPT{br}{jh}")
            for oh in range(PH):
                nc.tensor.matmul(
                    ps,
                    lhsT=W[(wk, oh)][:, jh * 128 : (jh + 1) * 128],
                    rhs=W[(wq, oh)],
                    start=(oh == 0),
                    stop=(oh == PH - 1),
                )
            t = consts.tile([128, DIM], FP16, tag=f"PT{br}{jh}", name=f"PT{br}{jh}")
            nc.vector.tensor_scalar_mul(out=t, in0=ps, scalar1=sign)
            PT[(br, jh)] = t
        ps = ps_pre.tile([128, 2 * PH], F32, tag="ps_pre", name=f"psUR{br}")
        for ih in range(PH):
            for oh in range(PH):
                nc.tensor.matmul(
                    ps[:, ih : ih + 1],
                    lhsT=W[(wq, oh)][:, ih * 128 : (ih + 1) * 128],
                    rhs=B[("bk_" + br, oh)],
                    start=(oh == 0),
                    stop=(oh == PH - 1),
                )
        for jh in range(PH):
            for oh in range(PH):
                nc.tensor.matmul(
                    ps[:, PH + jh : PH + jh + 1],
                    lhsT=W[(wk, oh)][:, jh * 128 : (jh + 1) * 128],
                    rhs=B[("bq_" + br, oh)],
                    start=(oh == 0),
                    stop=(oh == PH - 1),
                )
        tU = consts.tile([128, PH], F32, tag=f"U{br}", name=f"U{br}")
        nc.vector.tensor_scalar_mul(out=tU, in0=ps[:, 0:PH], scalar1=float(sign * N))
        tR = consts.tile([128, PH], FP16, tag=f"R{br}", name=f"R{br}")
        nc.vector.tensor_scalar_mul(out=tR, in0=ps[:, PH : 2 * PH], scalar1=sign)
        U[("full", br)] = tU
        R[("full", br)] = tR

    # batch-independent bias-dot part of c_diff: N*(bq_a.bk_a - bq_d.bk_d)
    ps = ps_pre.tile([1, 1], F32, tag="ps_pre", name="psCb")
    k = 0
    for bq, bk, sgn in (("bq_a", "bk_a", 1), ("bq_d", "bk_d", -1)):
        for oh in range(PH):
            t = consts.tile([128, 1], F32, tag=f"bkN{bk}{oh}", name=f"bkN{bk}{oh}")
            nc.vector.tensor_scalar_mul(out=t, in0=B[(bk, oh)], scalar1=float(sgn * N))
            nc.tensor.matmul(ps, lhsT=B[(bq, oh)], rhs=t, start=(k == 0), stop=(k == 3))
            k += 1
    c_bias = consts.tile([1, 1], FP16, tag="c_bias")
    nc.vector.tensor_scalar_mul(out=c_bias, in0=ps, scalar1=1.0)
    return PT, U, R, c_bias


def _body(tc, rgb, evt, wpack, bpack, out):
    nc = tc.nc
    ACT = mybir.ActivationFunctionType
    with ExitStack() as ctx:
        consts = ctx.enter_context(tc.tile_pool(name="consts", bufs=1))
        data = ctx.enter_context(tc.tile_pool(name="data", bufs=2))
        mpool = ctx.enter_context(tc.tile_pool(name="mpool", bufs=2))
        wbp = ctx.enter_context(tc.tile_pool(name="wbp", bufs=2))
        small = ctx.enter_context(tc.tile_pool(name="small", bufs=2))
        wrp = ctx.enter_context(tc.tile_pool(name="wrp", bufs=4))
        ps_val = ctx.enter_context(tc.tile_pool(name="ps_val", bufs=4, space="PSUM"))
        ps_pre = ctx.enter_context(tc.tile_pool(name="ps_pre", bufs=2, space="PSUM"))
        ps_head = ctx.enter_context(tc.tile_pool(name="ps_head", bufs=2, space="PSUM"))

        one_one = consts.tile([1, 1], FP16, tag="one_one")
        nc.vector.memset(one_one, 1.0)
        garbage = consts.tile([128, 1], F32, tag="garbage")

        def emit_weight_loads():
            # two contiguous f32 loads on the SP HWDGE queues: land early,
            # zero gpsimd cost, 128 large descriptors each
            W, B = {}, {}
            wt = consts.tile([128, 4 * PH * DIM], F32, tag="wpack")
            nc.sync.dma_start(out=wt, in_=wpack[:, :])
            bt = consts.tile([128, 4 * PH], F32, tag="bpack")
            nc.sync.dma_start(out=bt, in_=bpack[:, :])
            for wi, nm in enumerate(("Wq_a", "Wk_a", "Wq_d", "Wk_d")):
                for h in range(PH):
                    base = (wi * PH + h) * DIM
                    W[(nm, h)] = wt[:, base : base + DIM]
            for bi, nm in enumerate(("bq_a", "bk_a", "bq_d", "bk_d")):
                for h in range(PH):
                    B[(nm, h)] = bt[:, bi * PH + h : bi * PH + h + 1]
            return W, B

        st = [dict() for _ in range(BPC)]

        def emit_loads(b):
            # casting SWDGE loads: DRAM f32 -> SBUF fp16, one block at a time
            # in (blk, A/D x half) order matching downstream consumption.
            A, Dv = {}, {}
            for h in range(PH):
                A[h] = data.tile([128, N], FP16, tag=f"A{h}", name=f"A{h}_{b}")
                Dv[h] = data.tile([128, N], FP16, tag=f"D{h}", name=f"D{h}_{b}")
            for blk in range(NLB):
                sl = slice(blk * LB, (blk + 1) * LB)
                for h in range(PH):
                    nc.gpsimd.dma_start(out=A[h][:, sl], in_=rgb[b, h][:, sl])
                    nc.gpsimd.dma_start(out=Dv[h][:, sl], in_=evt[b, h][:, sl])
            st[b].update(A=A, Dv=Dv)

        def make_stage1(b):
            # per-block ops, returned as thunk lists in data-arrival order:
            #   red[i] : scalar row-sum partial of A     (blk-major, h-minor)
            #   sub[i] : DVE M = A - D with rowsum accum (blk-major, h-minor)
            A, Dv = st[b]["A"], st[b]["Dv"]
            M = {}
            sa4 = small.tile([128, 2 * NLB * PH], F32, tag="sa4", name=f"sa4_{b}")
            sm4 = small.tile([128, 2 * NLB * PH], F32, tag="sm4", name=f"sm4_{b}")
            sa16 = small.tile([128, NLB * PH], FP16, tag="sa16", name=f"sa16_{b}")
            sd16 = small.tile([128, NLB * PH], FP16, tag="sd16", name=f"sd16_{b}")
            for h in range(PH):
                M[h] = mpool.tile([128, N], FP16, tag=f"M{h}", name=f"M{h}_{b}")
            st[b].update(M=M, sa16=sa16, sd16=sd16)

            def red(h, blk):
                sl = slice(blk * LB, (blk + 1) * LB)
                nc.scalar.activation(
                    out=garbage.broadcast_to([128, LB]),
                    in_=A[h][:, sl],
                    func=ACT.Copy,
                    accum_out=sa4[:, blk * PH + h : blk * PH + h + 1],
                )

            def sub(h, blk):
                sl = slice(blk * LB, (blk + 1) * LB)
                nc.vector.scalar_tensor_tensor(
                    out=M[h][:, sl],
                    in0=A[h][:, sl],
                    scalar=1.0,
                    in1=Dv[h][:, sl],
                    op0=mybir.AluOpType.mult,
                    op1=mybir.AluOpType.subtract,
                    accum_out=sm4[:, blk * PH + h : blk * PH + h + 1],
                )

            def derive():
                # fp16 partials: sa16 = cast(sa4), sd16 = sa4 - sm4
                with nc.allow_low_precision(reason="tiny fp16 partials"):
                    nc.vector.tensor_scalar_mul(
                        out=sa16, in0=sa4[:, 0 : NLB * PH], scalar1=1.0
                    )
                    nc.vector.tensor_sub(
                        out=sd16, in0=sa4[:, 0 : NLB * PH], in1=sm4[:, 0 : NLB * PH]
                    )

            reds = [lambda h=h, blk=blk: red(h, blk) for blk in range(NLB) for h in range(PH)]
            subs = [lambda h=h, blk=blk: sub(h, blk) for blk in range(NLB) for h in range(PH)]
            return reds, subs, derive

        def head_pe(b):
            # c_diff = sum_j r[j] s[j] (both branches) + c_bias, and
            # v = PT @ s + U per branch -- all small PE matvecs
            sa16, sd16 = st[b]["sa16"], st[b]["sd16"]
            S4 = {"a": sa16, "d": sd16}
            ps_c = ps_head.tile([1, 1], F32, tag="ps_h", name=f"psc_{b}")
            terms = [
                (S4[br][:, blk * PH + jh : blk * PH + jh + 1], R[("full", br)][:, jh : jh + 1])
                for br in ("a", "d")
                for jh in range(PH)
                for blk in range(NLB)
            ]
            for i, (l, r) in enumerate(terms):
                nc.tensor.matmul(ps_c, lhsT=l, rhs=r, start=(i == 0), stop=False)
            nc.tensor.matmul(ps_c, lhsT=c_bias, rhs=one_one, start=False, stop=True)
            psv = {}
            for br in ("a", "d"):
                ps = ps_head.tile([128, PH], F32, tag="ps_h", name=f"psv{br}_{b}")
                for ih in range(PH):
                    k = 0
                    for jh in range(PH):
                        for blk in range(NLB):
                            nc.tensor.matmul(
                                ps[:, ih : ih + 1],
                                lhsT=PT[(br, jh)][:, ih * 128 : (ih + 1) * 128],
                                rhs=S4[br][:, blk * PH + jh : blk * PH + jh + 1],
                                start=(k == 0),
                                stop=(k == PH * NLB - 1),
                            )
                            k += 1
                psv[br] = ps
            st[b]["ps_c"], st[b]["ps_v"] = ps_c, psv

        def head_fin(b):
            # c16 on scalar; v add-U + fp16 cast on DVE
            c16 = small.tile([1, 1], F32, tag="c16", name=f"c16_{b}")
            nc.scalar.mul(out=c16, in_=st[b]["ps_c"], mul=INV_SQRT_D)
            VH = {}
            for br in ("a", "d"):
                v = small.tile([128, PH], F32, tag=f"v{br}", name=f"v{br}_{b}")
                nc.vector.tensor_add(out=v, in0=st[b]["ps_v"][br], in1=U[("full", br)])
                vh = small.tile([128, PH], FP16, tag=f"vh{br}", name=f"vh{br}_{b}")
                with nc.allow_low_precision(reason="fp16 matvec vector"):
                    nc.vector.tensor_scalar_mul(out=vh, in0=v, scalar1=1.0)
                VH[br] = vh
            st[b]["VH"], st[b]["c16"] = VH, c16

        def make_stage2(b):
            # per 512-chunk: PE 4 matvecs -> scalar sigmoid -> gpsimd bcast
            A, Dv = st[b]["A"], st[b]["Dv"]
            wb_sb = wbp.tile([128, N], FP16, tag="wb_sb", name=f"wb_sb_{b}")
            st[b]["wb_sb"] = wb_sb
            wrows = {}

            def chunk_pe(ich):
                VH = st[b]["VH"]
                sl = slice(ich * CH, (ich + 1) * CH)
                psv = ps_val.tile([1, CH], F32, tag="psv", name=f"psval{ich}_{b}")
                mms = [
                    (VH["a"][:, 0:1], A[0]), (VH["a"][:, 1:2], A[1]),
                    (VH["d"][:, 0:1], Dv[0]), (VH["d"][:, 1:2], Dv[1]),
                ]
                for i, (v, t) in enumerate(mms):
                    nc.tensor.matmul(
                        psv, lhsT=v, rhs=t[:, sl],
                        start=(i == 0), stop=(i == len(mms) - 1),
                    )
                wrows[ich] = psv

            def chunk_sig(ich):
                wrow = wrp.tile([1, CH], FP16, tag="wr", name=f"wrow{ich}_{b}")
                nc.scalar.activation(
                    out=wrow, in_=wrows[ich],
                    func=ACT.Sigmoid, bias=st[b]["c16"], scale=INV_SQRT_D,
                )
                wrows[ich] = wrow

            def chunk_bc(ich):
                nc.gpsimd.partition_broadcast(
                    wb_sb[:, ich * CH : (ich + 1) * CH], wrows[ich]
                )

            return chunk_pe, chunk_sig, chunk_bc

        def make_blend(b):
            # per (blk, h): DVE M *= wb ; A = M + D  (in place, all fp16 2x)
            A, Dv, M = st[b]["A"], st[b]["Dv"], st[b]["M"]

            def mul(h, blk):
                sl = slice(blk * LB, (blk + 1) * LB)
                wb_sb = st[b]["wb_sb"]
                nc.vector.tensor_mul(out=M[h][:, sl], in0=M[h][:, sl], in1=wb_sb[:, sl])

            def add(h, blk):
                sl = slice(blk * LB, (blk + 1) * LB)
                nc.vector.tensor_add(out=A[h][:, sl], in0=M[h][:, sl], in1=Dv[h][:, sl])

            def store(h, blk):
                sl = slice(blk * LB, (blk + 1) * LB)
                nc.gpsimd.dma_start(out=out[b, h][:, sl], in_=A[h][:, sl])

            return mul, add, store

        # ---- emission schedule ----------------------------------------
        emit_weight_loads_ret = emit_weight_loads()
        W, B = emit_weight_loads_ret
        emit_loads(0)
        emit_loads(1)
        PT, U, R, c_bias = _precompute(tc, consts, ps_pre, W, B)

        red0, sub0, derive0 = make_stage1(0)
        red1, sub1, derive1 = make_stage1(1)

        # scalar: reds in arrival order (b0 then b1 interleaved w/ sigmoids later)
        # DVE: subs b0
        for i in range(2 * NLB):
            red0[i]()
            sub0[i]()
        derive0()
        head_pe(0)
        head_fin(0)

        pe0, sig0, bc0 = make_stage2(0)
        mul0, add0, st0 = make_blend(0)
        pe1, sig1, bc1 = None, None, None

        # b1 stage-1 interleaved with b0 stage-2 in expected readiness order.
        # scalar stream: red1 blocks between sigmoid0 chunks
        # DVE stream: sub1 blocks around blend0 col-blocks
        # gpsimd stream: loads already queued; bcast0 chunks then stores
        red1[0]()                     # A0 blk0 of b1
        for ich in range(2):
            pe0(ich); sig0(ich); bc0(ich)
        red1[1]()                     # A1 blk0
        sub1[0](); sub1[1]()          # M blk0 (h0, h1)
        for ich in range(2, 4):
            pe0(ich); sig0(ich); bc0(ich)
        red1[2]()                     # A0 blk1
        # blend0 col-block 0 (stores emitted after bcast 4,5 to keep the
        # gpsimd stream in readiness order)
        mul0(0, 0); add0(0, 0)
        mul0(1, 0); add0(1, 0)
        for ich in range(4, 6):
            pe0(ich); sig0(ich); bc0(ich)
        st0(0, 0); st0(1, 0)
        red1[3]()                     # A1 blk1
        sub1[2](); sub1[3]()          # M blk1 (h0, h1)
        for ich in range(6, 8):
            pe0(ich); sig0(ich); bc0(ich)
        derive1()
        head_pe(1)
        head_fin(1)

        pe1, sig1, bc1 = make_stage2(1)
        mul1, add1, st1 = make_blend(1)

        # blend0 col-block 1 (wb0 complete) while b1 head/values spin up
        mul0(0, 1); add0(0, 1); st0(0, 1)
        mul0(1, 1); add0(1, 1); st0(1, 1)

        for ich in range(NCH):
            pe1(ich); sig1(ich); bc1(ich)
            if ich == 5:
                mul1(0, 0); add1(0, 0); st1(0, 0)
                mul1(1, 0); add1(1, 0); st1(1, 0)
        mul1(0, 1); add1(0, 1); st1(0, 1)
        mul1(1, 1); add1(1, 1); st1(1, 1)


_NC_CACHE = None


def _get_nc():
    global _NC_CACHE
    if _NC_CACHE is None:
        _NC_CACHE = build_nc()
    return _NC_CACHE


def _make_in_maps(inputs):
    rgb = np.ascontiguousarray(np.asarray(inputs["rgb"], dtype=np.float32)).reshape(
        BS, PH, 128, N
    )
    evt = np.ascontiguousarray(np.asarray(inputs["evt"], dtype=np.float32)).reshape(
        BS, PH, 128, N
    )
    wpack = np.stack(
        [
            np.asarray(inputs[nm], dtype=np.float32).reshape(PH, 128, DIM).transpose(1, 0, 2)
            for nm in ("Wq_a", "Wk_a", "Wq_d", "Wk_d")
        ],
        axis=1,
    ).reshape(128, 4 * PH * DIM)
    bpack = np.stack(
        [
            np.asarray(inputs[nm], dtype=np.float32).reshape(PH, 128).T
            for nm in ("bq_a", "bk_a", "bq_d", "bk_d")
        ],
        axis=1,
    ).reshape(128, 4 * PH)
    base = {"wpack": np.ascontiguousarray(wpack), "bpack": np.ascontiguousarray(bpack)}
    in_maps = []
    for c in range(NCORES):
        m = dict(base)
        m["rgb"] = np.ascontiguousarray(rgb[c * BPC : (c + 1) * BPC])
        m["evt"] = np.ascontiguousarray(evt[c * BPC : (c + 1) * BPC])
        in_maps.append(m)
    return in_maps


def run(inputs, trace=False):
    nc = _get_nc()
    in_maps = _make_in_maps(inputs)
    res = run_bass_kernel_spmd(nc, in_maps, core_ids=list(range(NCORES)), trace=trace)
    outs = [
        np.asarray(res.results[i]["out"]).reshape(BPC, DIM, HH, WW)
        for i in range(NCORES)
    ]
    full = np.concatenate(outs, axis=0)
    return full, res


def kernel(**inputs) -> np.ndarray:
    full, _ = run(inputs, trace=False)
    return full
